# revision 17
# baseline (speedup 1.0000x reference)
"""BinSAGE (2-layer GraphSAGE, mean aggregation, sign-binarized weights) on 8 TRN2 NeuronCores.

Strategy (graph/data parallel per the sharding hint):
  - dst nodes partitioned across 8 cores (tiles of 128 dst nodes; 50 tiles/core).
  - Edges sorted host-side by (dst_tile, src) and packed into 128-edge chunks.
    Each tile carries an "effective" chunk count = max needed across the 8
    cores (the SPMD program is shared), split into lo/hi classes because the
    bulk gather instruction (InstDMAGatherAnt) takes int16 row indices, so
    each gather table must stay under 32768 rows.
  - Node tables are stored REGION-REORDERED: region A = every core's first
    half (local row < span/2), region B = the second halves. This makes the
    lo/hi int16 split identical for layer 1 (x) and layer 2 (y2), and lets
    the inter-layer AllGather be split in two halves that land directly as
    regions A and B — the first half overlaps with layer-1 compute.
  - Features live in HBM as bf16 rows padded to 256 B (dma_gather granularity).
    Gathers stream in <=1024-index dma_gather calls round-robined across 4
    SWDGE queues (desc-gen parallelizes almost linearly across queues)
    through rotating SBUF buffers.
  - Per tile: ALL chunk one-hots are built in two batched DVE ops using
    stride-0 broadcast APs: oh[:, j, :] = (iota == dloc_j) * rdeg_j, where
    rdeg_j is 1/max(deg(dst),1) of each edge slot — the mean scaling is
    folded into the one-hot. Pad slots carry dloc=384 so they contribute 0.
  - Per chunk: a TensorEngine matmul accumulates segment MEANS into PSUM as
    aggT [F, 128dst]; one ScalarEngine copy casts PSUM->SBUF bf16 into a
    buffer with a preset all-ones row (bias folded via an extra weight row).
  - Layer 2 is transform-first: y2 = h @ sign(w2_l).T computed per shard,
    all-gathered in two region halves, then aggregated exactly like layer 1
    (same index streams).
  - Weights/biases are binarized + transposed on host (tiny, replicated).
"""

import numpy as np
import ml_dtypes

import concourse.bass as bass
import concourse.bacc as bacc
import concourse.mybir as mybir
import concourse.tile as tile
from concourse import bass_utils

BF16 = ml_dtypes.bfloat16
P = 128            # partitions == dst-tile width == edge-chunk size
N_CORES = 8
ROW = 128          # padded feature row (bf16) -> 256 B, dma_gather granularity
GC = 8             # chunks per dma_gather call (1024 idxs = ucode scratch cap)
NQ = 4             # SWDGE queues (ucode max) — desc-gen parallelism


class Cfg:
    def __init__(self, n_nodes, in_dim, hid, out_dim, tiles_per_core):
        self.n_nodes = n_nodes
        self.in_dim = in_dim
        self.hid = hid
        self.out_dim = out_dim
        self.tiles_per_core = tiles_per_core
        self.span = tiles_per_core * P           # dst nodes per core
        self.hs = self.span // 2                 # region half-span per core
        self.n_pad = self.span * N_CORES         # padded global node count
        self.reg = self.hs * N_CORES             # rows per region table
        assert self.n_pad >= n_nodes
        assert self.reg <= 32767                 # int16 gather indices


FULL_CFG = Cfg(n_nodes=50000, in_dim=96, hid=128, out_dim=64, tiles_per_core=50)


def _wrap16(v):
    """Pack an int16 stream v (len % 16 == 0) into the [128, len/16] SBUF
    layout dma_gather expects: element i at [i % 16, i // 16], replicated
    into each of the 8 Q7-core partition groups (HW-verified convention)."""
    n = len(v)
    return np.ascontiguousarray(np.tile(v.reshape(n // 16, 16).T, (8, 1)))


class Sched:
    """Host-computed, core-uniform chunk schedule (shared by both layers)."""
    def __init__(self, eff_kl, eff_kh):
        self.eff_kl = eff_kl                     # per-tile lo chunks
        self.eff_kh = eff_kh                     # per-tile hi chunks
        self.off_lo = np.zeros(len(eff_kl) + 1, np.int64)
        self.off_lo[1:] = np.cumsum(eff_kl)
        self.off_hi = np.zeros(len(eff_kh) + 1, np.int64)
        self.off_hi[1:] = np.cumsum(eff_kh)
        self.off_d = np.zeros(len(eff_kl) + 1, np.int64)
        self.off_d[1:] = np.cumsum(eff_kl + eff_kh)
        self.SL = int(self.off_lo[-1])           # total lo chunks per core
        self.SH = int(self.off_hi[-1])           # total hi chunks per core
        self.SD = int(self.off_d[-1])            # total dloc columns
        self.KMAX = int((eff_kl + eff_kh).max()) # chunks in widest tile


def preprocess(x, edge_index, w1_l, b1, w1_r, w2_l, b2, w2_r, cfg):
    """Host-side sharding/layout. Returns (in_maps, sched)."""
    src = np.asarray(edge_index[0]).astype(np.int64)
    dst = np.asarray(edge_index[1]).astype(np.int64)
    n_tiles_total = N_CORES * cfg.tiles_per_core
    tpc = cfg.tiles_per_core

    g = dst // P                                  # global dst-tile id
    order = np.lexsort((src, g))                  # tile-grouped, src-sorted
    src_s = src[order]
    g_s = g[order]
    dloc_s = (dst[order] % P).astype(np.float32)

    # region mapping: core c local r -> A: c*hs + r | B: c*hs + (r - hs)
    c_s = src_s // cfg.span
    r_s = src_s % cfg.span
    lo = r_s < cfg.hs
    regidx = np.where(lo, c_s * cfg.hs + r_s, c_s * cfg.hs + (r_s - cfg.hs))

    cnt = np.bincount(g_s, minlength=n_tiles_total).astype(np.int64)
    cnt_lo = np.bincount(g_s[lo], minlength=n_tiles_total).astype(np.int64)
    cnt_hi = cnt - cnt_lo

    # effective chunk counts per LOCAL tile = max over the 8 cores
    eff_kl = np.ceil(cnt_lo.reshape(N_CORES, tpc).max(axis=0) / P).astype(np.int64)
    eff_kh = np.ceil(cnt_hi.reshape(N_CORES, tpc).max(axis=0) / P).astype(np.int64)
    eff_kl[(eff_kl == 0) & (eff_kh == 0)] = 1    # keep PSUM written on pad tiles
    sched = Sched(eff_kl, eff_kh)

    # edges are sorted by src within the tile, so lo/hi interleave; compact
    # each class by its own running position.
    lo_rank = np.cumsum(lo) - 1
    hi_rank = np.cumsum(~lo) - 1
    tile_lo_start = np.zeros(n_tiles_total, np.int64)
    tile_hi_start = np.zeros(n_tiles_total, np.int64)
    np.cumsum(cnt_lo[:-1], out=tile_lo_start[1:])
    np.cumsum(cnt_hi[:-1], out=tile_hi_start[1:])
    poslo = lo_rank[lo] - tile_lo_start[g_s[lo]]
    poshi = hi_rank[~lo] - tile_hi_start[g_s[~lo]]

    # reciprocal degrees, pre-broadcast down the feature partitions so the
    # mean is one DVE mult per tile (rows all identical)
    deg = np.bincount(dst, minlength=cfg.n_pad).astype(np.float32)
    rdeg = (1.0 / np.maximum(deg, 1.0)).astype(np.float32)
    rdeg_pc = [np.ascontiguousarray(np.broadcast_to(
        rdeg[None, c * cfg.span:(c + 1) * cfg.span],
        (cfg.in_dim, cfg.span))).astype(BF16) for c in range(N_CORES)]

    # per-tile slot arrays at the max width, then compact per-tile
    KLm, KHm = int(max(eff_kl.max(), 1)), int(max(eff_kh.max(), 1))
    idxlo = np.zeros((n_tiles_total, KLm * P), dtype=np.int16)
    idxhi = np.zeros((n_tiles_total, KHm * P), dtype=np.int16)
    dlo = np.full((n_tiles_total, KLm * P), 384.0, dtype=np.float32)
    dhi = np.full((n_tiles_total, KHm * P), 384.0, dtype=np.float32)
    idxlo[g_s[lo], poslo] = regidx[lo].astype(np.int16)
    idxhi[g_s[~lo], poshi] = regidx[~lo].astype(np.int16)
    dlo[g_s[lo], poslo] = dloc_s[lo]
    dhi[g_s[~lo], poshi] = dloc_s[~lo]

    # compacted per-core streams in (tile, chunk, partition) order
    idxlo_pc, idxhi_pc, dloc_pc = [], [], []
    for c in range(N_CORES):
        lo_parts, hi_parts, d_parts = [], [], []
        for t in range(tpc):
            gt = c * tpc + t
            nl, nh = int(eff_kl[t]), int(eff_kh[t])
            lo_parts.append(idxlo[gt, : nl * P])
            hi_parts.append(idxhi[gt, : nh * P])
            d_parts.append(dlo[gt, : nl * P].reshape(nl, P))
            d_parts.append(dhi[gt, : nh * P].reshape(nh, P))
        idxlo_pc.append(_wrap16(np.concatenate(lo_parts)))
        idxhi_pc.append(_wrap16(
            np.concatenate(hi_parts) if sched.SH else np.zeros(P, np.int16)))
        # dloc: [SD chunks, P] -> [P, SD] bf16
        dloc_pc.append(np.ascontiguousarray(
            np.concatenate(d_parts, axis=0).T).astype(BF16))

    # region-reordered bf16 feature tables (256B rows)
    xpad = np.zeros((cfg.n_pad, ROW), dtype=BF16)
    xpad[: cfg.n_nodes, : cfg.in_dim] = np.asarray(x, np.float32)
    x3 = xpad.reshape(N_CORES, cfg.span, ROW)
    tab_a = np.ascontiguousarray(x3[:, : cfg.hs].reshape(cfg.reg, ROW))
    tab_b = np.ascontiguousarray(x3[:, cfg.hs:].reshape(cfg.reg, ROW))

    # per-core transposed x slice for the self (lin_r) term
    xt_pc = [
        np.ascontiguousarray(xpad[c * cfg.span:(c + 1) * cfg.span,
                                  : cfg.in_dim].T)
        for c in range(N_CORES)
    ]

    sgn = lambda w: np.sign(np.asarray(w, dtype=np.float32))
    w1lt = np.concatenate([sgn(w1_l).T, np.asarray(b1, np.float32)[None, :]],
                          0).astype(BF16)
    w1rt = np.ascontiguousarray(sgn(w1_r).T).astype(BF16)
    w2lt = np.ascontiguousarray(sgn(w2_l).T).astype(BF16)
    w2rt = np.ascontiguousarray(sgn(w2_r).T).astype(BF16)
    ib2 = np.concatenate(
        [np.eye(cfg.out_dim, dtype=np.float32),
         np.asarray(b2, np.float32)[None, :]], 0).astype(BF16)

    in_maps = []
    for c in range(N_CORES):
        in_maps.append({
            "taba": tab_a, "tabb": tab_b,
            "xt": xt_pc[c],
            "idxlo": idxlo_pc[c], "idxhi": idxhi_pc[c],
            "dloc": dloc_pc[c], "rdeg": rdeg_pc[c],
            "w1lt": w1lt, "w1rt": w1rt, "w2lt": w2lt, "w2rt": w2rt, "ib2": ib2,
        })
    return in_maps, sched


def build_program(cfg, sched, enable_asserts=False):
    tpc = cfg.tiles_per_core
    NBUF = 14                                     # rotating gather-call buffers
    NB = 3                                        # small persistent buffer depth
    SL, SH, SD = sched.SL, sched.SH, sched.SD
    KMAX = sched.KMAX

    dt = mybir.dt
    f32, bf, i16 = dt.float32, dt.bfloat16, dt.int16
    IN, HID, OUT = cfg.in_dim, cfg.hid, cfg.out_dim

    nc = bacc.Bacc("TRN2", target_bir_lowering=False, debug=False,
                   enable_asserts=enable_asserts, num_devices=N_CORES,
                   num_swdge_queues=NQ)

    taba = nc.dram_tensor("taba", [cfg.reg, ROW], bf, kind="ExternalInput")
    tabb = nc.dram_tensor("tabb", [cfg.reg, ROW], bf, kind="ExternalInput")
    xt = nc.dram_tensor("xt", [IN, cfg.span], bf, kind="ExternalInput")
    idxlo = nc.dram_tensor("idxlo", [P, SL * 8], i16, kind="ExternalInput")
    idxhi = nc.dram_tensor("idxhi", [P, max(SH, 1) * 8], i16,
                           kind="ExternalInput")
    dloc = nc.dram_tensor("dloc", [P, SD], bf, kind="ExternalInput")
    rdeg = nc.dram_tensor("rdeg", [cfg.in_dim, cfg.span], bf,
                          kind="ExternalInput")
    w1lt = nc.dram_tensor("w1lt", [IN + 1, HID], bf, kind="ExternalInput")
    w1rt = nc.dram_tensor("w1rt", [IN, HID], bf, kind="ExternalInput")
    w2lt = nc.dram_tensor("w2lt", [HID, OUT], bf, kind="ExternalInput")
    w2rt = nc.dram_tensor("w2rt", [HID, OUT], bf, kind="ExternalInput")
    ib2 = nc.dram_tensor("ib2", [OUT + 1, OUT], bf, kind="ExternalInput")
    outd = nc.dram_tensor("out", [cfg.span, OUT], f32, kind="ExternalOutput")

    AF = mybir.ActivationFunctionType
    OP = mybir.AluOpType

    with tile.TileContext(nc) as tc:
        with tc.tile_pool(name="res", bufs=1) as res, \
             tc.tile_pool(name="msgp", bufs=1) as msgp, \
             tc.tile_pool(name="ohp", bufs=3) as ohp, \
             tc.tile_pool(name="xtp", bufs=3) as xtp, \
             tc.tile_pool(name="scp", bufs=3) as scp, \
             tc.tile_pool(name="ps_agg", bufs=2, space="PSUM") as ps_agg, \
             tc.tile_pool(name="ps_o", bufs=2, space="PSUM") as ps_o, \
             tc.tile_pool(name="ps_y", bufs=2, space="PSUM") as ps_y, \
             tc.tile_pool(name="dramp", bufs=1, space="DRAM") as dramp:

            # ---------------- resident data ----------------
            idxlo_sb = res.tile([P, SL * 8], i16, name="idxlo_sb")
            nc.sync.dma_start(idxlo_sb[:], idxlo[:])
            idxhi_sb = res.tile([P, max(SH, 1) * 8], i16, name="idxhi_sb")
            nc.sync.dma_start(idxhi_sb[:], idxhi[:])
            dloc_sb = res.tile([P, SD], bf, name="dloc_sb")
            nc.sync.dma_start(dloc_sb[:], dloc[:])
            rdeg_sb = res.tile([cfg.in_dim, cfg.span], bf, name="rdeg_sb")
            nc.sync.dma_start(rdeg_sb[:], rdeg[:])
            w1lt_sb = res.tile([IN + 1, HID], bf, name="w1lt_sb")
            nc.sync.dma_start(w1lt_sb[:], w1lt[:])
            w1rt_sb = res.tile([IN, HID], bf, name="w1rt_sb")
            nc.sync.dma_start(w1rt_sb[:], w1rt[:])
            w2lt_sb = res.tile([HID, OUT], bf, name="w2lt_sb")
            nc.sync.dma_start(w2lt_sb[:], w2lt[:])
            w2rt_sb = res.tile([HID, OUT], bf, name="w2rt_sb")
            nc.sync.dma_start(w2rt_sb[:], w2rt[:])
            ib2_sb = res.tile([OUT + 1, OUT], bf, name="ib2_sb")
            nc.sync.dma_start(ib2_sb[:], ib2[:])

            iota_i = res.tile([P, P], mybir.dt.int32, name="iota_i")
            nc.gpsimd.iota(iota_i[:], pattern=[[1, P]], base=0,
                           channel_multiplier=0)
            iota_bf = res.tile([P, P], bf, name="iota_bf")
            nc.vector.tensor_copy(iota_bf[:], iota_i[:])

            ht_tiles = [res.tile([HID, P], bf, name=f"ht{t}")
                        for t in range(tpc)]

            # persistent gather-call buffers (m_lo/m_hi shared by layer 1 and
            # phase 2b; m_lo2 is separate so phase 2a can overlap layer 1)
            m_lo = [msgp.tile([P, GC, ROW], bf, name=f"mlo{i}")
                    for i in range(NBUF)]
            m_hi = [msgp.tile([P, GC, ROW], bf, name=f"mhi{i}")
                    for i in range(NBUF)]
            m_lo2 = [msgp.tile([P, GC, ROW], bf, name=f"mlo2_{i}")
                     for i in range(NBUF)]
            # persistent agg tiles (aggs1 carries the all-ones bias row)
            aggs1 = [msgp.tile([IN + 1, P], bf, name=f"aggs1_{i}")
                     for i in range(NB)]
            aggs2 = [msgp.tile([OUT, P], bf, name=f"aggs2_{i}")
                     for i in range(NB)]
            y2sbs = [msgp.tile([P, ROW], bf, name=f"y2sb{i}")
                     for i in range(NB)]
            for i in range(NB):
                nc.vector.memset(aggs1[i][IN:IN + 1, :], 1.0)
                nc.vector.memset(y2sbs[i][:, OUT:ROW], 0.0)
            # per-tile lo-class partial means for layer 2 (ones row for bias)
            pl_tiles = [msgp.tile([OUT + 1, P], bf, name=f"pl{t}")
                        for t in range(tpc)]
            for t in range(tpc):
                nc.vector.memset(pl_tiles[t][0:OUT, :], 0.0)
                nc.vector.memset(pl_tiles[t][OUT:OUT + 1, :], 1.0)

            y2in = dramp.tile([cfg.span, ROW], bf, name="y2in")
            y2fa = dramp.tile([cfg.reg, ROW], bf, name="y2fa",
                              addr_space="Shared")
            y2fb = dramp.tile([cfg.reg, ROW], bf, name="y2fb",
                              addr_space="Shared")

            qctr = [0]                            # SWDGE queue round-robin

            def make_stream(bufs, idx_sb, tab, total):
                state = [0]

                def ensure(upto_call):
                    while state[0] <= upto_call:
                        c = state[0]
                        ncall = min(GC, total - c * GC)
                        num = ncall * P
                        dest = bufs[c % NBUF]
                        nc.gpsimd.dma_gather(
                            out_ap=dest[:, 0:ncall, :],
                            in_ap=tab,
                            idxs_ap=idx_sb[:, c * (GC * 8):
                                           c * (GC * 8) + num // 16],
                            num_idxs=num,
                            num_idxs_reg=num,
                            elem_size=ROW,
                            queue_num=qctr[0] % NQ,
                        )
                        qctr[0] += 1
                        state[0] += 1
                return ensure

            def build_oh(oh, col0, n):
                """Batched one-hots for n chunk columns starting at col0:
                oh[:, j, :] = (iota == dloc_j)."""
                iota_b = iota_bf[:].unsqueeze(1).broadcast_to([P, n, P])
                dloc_b = dloc_sb[:, col0:col0 + n].unsqueeze(2) \
                    .broadcast_to([P, n, P])
                nc.vector.tensor_tensor(oh[:, 0:n, :], iota_b, dloc_b,
                                        OP.is_equal)

            def mean_scale(out_ap, agg, F_agg, t):
                """out = agg * (1/deg): one DVE mult against the resident
                pre-broadcast rdeg slab."""
                nc.vector.tensor_tensor(
                    out_ap, agg[:], rdeg_sb[0:F_agg, t * P:(t + 1) * P],
                    OP.mult)

            off_lo, off_hi = sched.off_lo, sched.off_hi
            eff_l, eff_h = sched.eff_kl, sched.eff_kh
            off_d = sched.off_d

            def call_hi(arr_off, arr_eff, t):
                return (int(arr_off[t]) + int(arr_eff[t]) - 1) // GC

            # ---------------- layer 1 (+ y2 projection) ----------------
            ens_lo1 = make_stream(m_lo, idxlo_sb, taba[:], SL)
            ens_hi1 = make_stream(m_hi, idxhi_sb, tabb[:], SH)
            for t in range(tpc):
                tp = min(t + 1, tpc - 1)           # prefetch one tile ahead
                if SL:
                    ens_lo1(call_hi(off_lo, eff_l, tp))
                if SH:
                    ens_hi1(call_hi(off_hi, eff_h, tp))
                nl, nh = int(eff_l[t]), int(eff_h[t])
                oh = ohp.tile([P, KMAX, P], bf, tag="oh")
                build_oh(oh, int(off_d[t]), nl + nh)
                agg = ps_agg.tile([IN, P], f32, tag="agg")
                chunks = [(m_lo, int(off_lo[t]) + k) for k in range(nl)]
                chunks += [(m_hi, int(off_hi[t]) + k) for k in range(nh)]
                for j, (bufs, cpos) in enumerate(chunks):
                    mb = bufs[(cpos // GC) % NBUF]
                    nc.tensor.matmul(
                        out=agg[:], lhsT=mb[:, cpos % GC, 0:IN],
                        rhs=oh[:, j, :], start=(j == 0),
                        stop=(j == len(chunks) - 1))
                ab = aggs1[t % NB]
                mean_scale(ab[0:IN, :], agg, IN, t)
                # tail: h = relu(w1l @ [agg;1] + w1r @ x_self); y2 = h @ w2l
                xt_t = xtp.tile([IN, P], bf, tag="xt")
                nc.sync.dma_start(xt_t[:], xt[:, t * P:(t + 1) * P])
                hps = ps_o.tile([HID, P], f32, tag="hps")
                nc.tensor.matmul(out=hps[:], lhsT=w1lt_sb[:], rhs=ab[:],
                                 start=True, stop=False)
                nc.tensor.matmul(out=hps[:], lhsT=w1rt_sb[:], rhs=xt_t[:],
                                 start=False, stop=True)
                nc.scalar.activation(out=ht_tiles[t][:], in_=hps[:],
                                     func=AF.Relu)
                y2ps = ps_y.tile([P, OUT], f32, tag="y2ps")
                nc.tensor.matmul(out=y2ps[:], lhsT=ht_tiles[t][:],
                                 rhs=w2lt_sb[:], start=True, stop=True)
                ysb = y2sbs[t % NB]
                nc.scalar.activation(out=ysb[:, 0:OUT], in_=y2ps[:],
                                     func=AF.Copy)
                nc.sync.dma_start(y2in[t * P:(t + 1) * P, :], ysb[:])

            # AllGathers for both regions, back to back after layer 1; the
            # region-A phase (2a) only waits on the first
            nc.gpsimd.collective_compute(
                "AllGather", OP.bypass,
                replica_groups=[list(range(N_CORES))],
                ins=[y2in[0:cfg.hs, :].opt()], outs=[y2fa.opt()],
            )
            nc.gpsimd.collective_compute(
                "AllGather", OP.bypass,
                replica_groups=[list(range(N_CORES))],
                ins=[y2in[cfg.hs:cfg.span, :].opt()],
                outs=[y2fb.opt()],
            )

            # ------------- layer 2a: lo-class partial means --------------
            # only needs region A; AG#2 (region B) is emitted mid-phase so
            # its Pool block overlaps 2a compute
            ens_lo2 = make_stream(m_lo2, idxlo_sb, y2fa[:], SL)
            for t in range(tpc):
                tp = min(t + 1, tpc - 1)
                if SL:
                    ens_lo2(call_hi(off_lo, eff_l, tp))
                nl = int(eff_l[t])
                if nl == 0:
                    continue
                oh = ohp.tile([P, KMAX, P], bf, tag="oh")
                build_oh(oh, int(off_d[t]), nl)
                agg = ps_agg.tile([OUT, P], f32, tag="agg")
                for k in range(nl):
                    cpos = int(off_lo[t]) + k
                    mb = m_lo2[(cpos // GC) % NBUF]
                    nc.tensor.matmul(
                        out=agg[:], lhsT=mb[:, cpos % GC, 0:OUT],
                        rhs=oh[:, k, :], start=(k == 0), stop=(k == nl - 1))
                mean_scale(pl_tiles[t][0:OUT, :], agg, OUT, t)

            # ------------- layer 2b: hi-class + combine + output ---------
            ens_hi2 = make_stream(m_hi, idxhi_sb, y2fb[:], SH)
            for t in range(tpc):
                tp = min(t + 1, tpc - 1)
                if SH:
                    ens_hi2(call_hi(off_hi, eff_h, tp))
                nh = int(eff_h[t])
                ops_ = ps_o.tile([P, OUT], f32, tag="hps")
                nc.tensor.matmul(out=ops_[:], lhsT=ht_tiles[t][:],
                                 rhs=w2rt_sb[:], start=True, stop=False)
                if nh:
                    oh = ohp.tile([P, KMAX, P], bf, tag="oh")
                    build_oh(oh, int(off_d[t]) + int(eff_l[t]), nh)
                    agg = ps_agg.tile([OUT, P], f32, tag="agg")
                    for k in range(nh):
                        cpos = int(off_hi[t]) + k
                        mb = m_hi[(cpos // GC) % NBUF]
                        nc.tensor.matmul(
                            out=agg[:], lhsT=mb[:, cpos % GC, 0:OUT],
                            rhs=oh[:, k, :], start=(k == 0),
                            stop=(k == nh - 1))
                    ab = aggs2[t % NB]
                    mean_scale(ab[:], agg, OUT, t)
                    nc.tensor.matmul(out=ops_[:], lhsT=ab[:],
                                     rhs=ib2_sb[0:OUT, :],
                                     start=False, stop=False)
                nc.tensor.matmul(out=ops_[:], lhsT=pl_tiles[t][:],
                                 rhs=ib2_sb[:], start=False, stop=True)
                osb = scp.tile([P, OUT], f32, tag="osb")
                nc.scalar.activation(out=osb[:], in_=ops_[:], func=AF.Copy)
                nc.sync.dma_start(outd[t * P:(t + 1) * P, :], osb[:])

    nc.compile()
    return nc


def run(inputs, cfg, trace=False):
    in_maps, sched = preprocess(cfg=cfg, **inputs)
    nc = build_program(cfg, sched)
    res = bass_utils.run_bass_kernel_spmd(
        nc, in_maps, list(range(N_CORES)), trace=trace)
    outs = [res.results[c]["out"] for c in range(N_CORES)]
    full = np.concatenate(outs, axis=0)[: cfg.n_nodes]
    return np.ascontiguousarray(full.astype(np.float32)), res


def kernel(**inputs):
    out, _ = run(inputs, FULL_CFG, trace=False)
    return out


# revision 19
# speedup vs baseline: 1.1740x; 1.1740x over previous
"""BinSAGE (2-layer GraphSAGE, mean aggregation, sign-binarized weights) on 8 TRN2 NeuronCores.

Strategy (graph/data parallel per the sharding hint):
  - dst nodes partitioned across 8 cores (tiles of 128 dst nodes; 50 tiles/core).
  - Edges sorted host-side by (dst_tile, src) and packed into 128-edge chunks.
    Each tile carries an "effective" chunk count = max needed across the 8
    cores (the SPMD program is shared), split into lo/hi classes because the
    bulk gather instruction (InstDMAGatherAnt) takes int16 row indices, so
    each gather table must stay under 32768 rows.
  - Node tables are stored REGION-REORDERED: region A = every core's first
    half (local row < span/2), region B = the second halves. This makes the
    lo/hi int16 split identical for layer 1 (x) and layer 2 (y2), and lets
    the inter-layer AllGather be split in two halves that land directly as
    regions A and B — the first half overlaps with layer-1 compute.
  - Features live in HBM as bf16 rows padded to 256 B (dma_gather granularity).
    Gathers stream in <=1024-index dma_gather calls round-robined across 4
    SWDGE queues (desc-gen parallelizes almost linearly across queues)
    through rotating SBUF buffers.
  - Per tile: ALL chunk one-hots are built in two batched DVE ops using
    stride-0 broadcast APs: oh[:, j, :] = (iota == dloc_j) * rdeg_j, where
    rdeg_j is 1/max(deg(dst),1) of each edge slot — the mean scaling is
    folded into the one-hot. Pad slots carry dloc=384 so they contribute 0.
  - Per chunk: a TensorEngine matmul accumulates segment MEANS into PSUM as
    aggT [F, 128dst]; one ScalarEngine copy casts PSUM->SBUF bf16 into a
    buffer with a preset all-ones row (bias folded via an extra weight row).
  - Layer 2 is transform-first: y2 = h @ sign(w2_l).T computed per shard,
    all-gathered in two region halves, then aggregated exactly like layer 1
    (same index streams).
  - Weights/biases are binarized + transposed on host (tiny, replicated).
"""

import numpy as np
import ml_dtypes

import concourse.bass as bass
import concourse.bacc as bacc
import concourse.mybir as mybir
import concourse.tile as tile
from concourse import bass_utils

BF16 = ml_dtypes.bfloat16
P = 128            # partitions == dst-tile width == edge-chunk size
N_CORES = 8
ROW = 128          # padded feature row (bf16) -> 256 B, dma_gather granularity
GC = 8             # chunks per dma_gather call (1024 idxs = ucode scratch cap)
NQ = 4             # SWDGE queues (ucode max) — desc-gen parallelism


class Cfg:
    def __init__(self, n_nodes, in_dim, hid, out_dim, tiles_per_core):
        self.n_nodes = n_nodes
        self.in_dim = in_dim
        self.hid = hid
        self.out_dim = out_dim
        self.tiles_per_core = tiles_per_core
        self.span = tiles_per_core * P           # dst nodes per core
        self.hs = self.span // 2                 # region half-span per core
        self.n_pad = self.span * N_CORES         # padded global node count
        self.reg = self.hs * N_CORES             # rows per region table
        assert self.n_pad >= n_nodes
        assert self.reg <= 32767                 # int16 gather indices


FULL_CFG = Cfg(n_nodes=50000, in_dim=96, hid=128, out_dim=64, tiles_per_core=50)


def _wrap16(v):
    """Pack an int16 stream v (len % 16 == 0) into the [128, len/16] SBUF
    layout dma_gather expects: element i at [i % 16, i // 16], replicated
    into each of the 8 Q7-core partition groups (HW-verified convention)."""
    n = len(v)
    return np.ascontiguousarray(np.tile(v.reshape(n // 16, 16).T, (8, 1)))


class Sched:
    """Host-computed, core-uniform chunk schedule (shared by both layers)."""
    def __init__(self, eff_kl, eff_kh):
        self.eff_kl = eff_kl                     # per-tile lo chunks
        self.eff_kh = eff_kh                     # per-tile hi chunks
        self.off_lo = np.zeros(len(eff_kl) + 1, np.int64)
        self.off_lo[1:] = np.cumsum(eff_kl)
        self.off_hi = np.zeros(len(eff_kh) + 1, np.int64)
        self.off_hi[1:] = np.cumsum(eff_kh)
        self.off_d = np.zeros(len(eff_kl) + 1, np.int64)
        self.off_d[1:] = np.cumsum(eff_kl + eff_kh)
        self.SL = int(self.off_lo[-1])           # total lo chunks per core
        self.SH = int(self.off_hi[-1])           # total hi chunks per core
        self.SD = int(self.off_d[-1])            # total dloc columns
        self.KMAX = int((eff_kl + eff_kh).max()) # chunks in widest tile


def preprocess(x, edge_index, w1_l, b1, w1_r, w2_l, b2, w2_r, cfg):
    """Host-side sharding/layout. Returns (in_maps, sched)."""
    src = np.asarray(edge_index[0]).astype(np.int64)
    dst = np.asarray(edge_index[1]).astype(np.int64)
    n_tiles_total = N_CORES * cfg.tiles_per_core
    tpc = cfg.tiles_per_core

    g = dst // P                                  # global dst-tile id
    order = np.lexsort((src, g))                  # tile-grouped, src-sorted
    src_s = src[order]
    g_s = g[order]
    dloc_s = (dst[order] % P).astype(np.float32)

    # region mapping: core c local r -> A: c*hs + r | B: c*hs + (r - hs)
    c_s = src_s // cfg.span
    r_s = src_s % cfg.span
    lo = r_s < cfg.hs
    regidx = np.where(lo, c_s * cfg.hs + r_s, c_s * cfg.hs + (r_s - cfg.hs))

    cnt = np.bincount(g_s, minlength=n_tiles_total).astype(np.int64)
    cnt_lo = np.bincount(g_s[lo], minlength=n_tiles_total).astype(np.int64)
    cnt_hi = cnt - cnt_lo

    # effective chunk counts per LOCAL tile = max over the 8 cores
    eff_kl = np.ceil(cnt_lo.reshape(N_CORES, tpc).max(axis=0) / P).astype(np.int64)
    eff_kh = np.ceil(cnt_hi.reshape(N_CORES, tpc).max(axis=0) / P).astype(np.int64)
    eff_kl[(eff_kl == 0) & (eff_kh == 0)] = 1    # keep PSUM written on pad tiles
    sched = Sched(eff_kl, eff_kh)

    # edges are sorted by src within the tile, so lo/hi interleave; compact
    # each class by its own running position.
    lo_rank = np.cumsum(lo) - 1
    hi_rank = np.cumsum(~lo) - 1
    tile_lo_start = np.zeros(n_tiles_total, np.int64)
    tile_hi_start = np.zeros(n_tiles_total, np.int64)
    np.cumsum(cnt_lo[:-1], out=tile_lo_start[1:])
    np.cumsum(cnt_hi[:-1], out=tile_hi_start[1:])
    poslo = lo_rank[lo] - tile_lo_start[g_s[lo]]
    poshi = hi_rank[~lo] - tile_hi_start[g_s[~lo]]

    # reciprocal degrees, pre-broadcast down the feature partitions so the
    # mean is one DVE mult per tile (rows all identical)
    deg = np.bincount(dst, minlength=cfg.n_pad).astype(np.float32)
    rdeg = (1.0 / np.maximum(deg, 1.0)).astype(np.float32)
    rdeg_pc = [np.ascontiguousarray(np.broadcast_to(
        rdeg[None, c * cfg.span:(c + 1) * cfg.span],
        (cfg.in_dim, cfg.span))).astype(BF16) for c in range(N_CORES)]

    # per-tile slot arrays at the max width, then compact per-tile
    KLm, KHm = int(max(eff_kl.max(), 1)), int(max(eff_kh.max(), 1))
    idxlo = np.zeros((n_tiles_total, KLm * P), dtype=np.int16)
    idxhi = np.zeros((n_tiles_total, KHm * P), dtype=np.int16)
    dlo = np.full((n_tiles_total, KLm * P), 384.0, dtype=np.float32)
    dhi = np.full((n_tiles_total, KHm * P), 384.0, dtype=np.float32)
    idxlo[g_s[lo], poslo] = regidx[lo].astype(np.int16)
    idxhi[g_s[~lo], poshi] = regidx[~lo].astype(np.int16)
    dlo[g_s[lo], poslo] = dloc_s[lo]
    dhi[g_s[~lo], poshi] = dloc_s[~lo]

    # compacted per-core streams in (tile, chunk, partition) order
    idxlo_pc, idxhi_pc, dloc_pc = [], [], []
    for c in range(N_CORES):
        lo_parts, hi_parts, d_parts = [], [], []
        for t in range(tpc):
            gt = c * tpc + t
            nl, nh = int(eff_kl[t]), int(eff_kh[t])
            lo_parts.append(idxlo[gt, : nl * P])
            hi_parts.append(idxhi[gt, : nh * P])
            d_parts.append(dlo[gt, : nl * P].reshape(nl, P))
            d_parts.append(dhi[gt, : nh * P].reshape(nh, P))
        idxlo_pc.append(_wrap16(np.concatenate(lo_parts)))
        idxhi_pc.append(_wrap16(
            np.concatenate(hi_parts) if sched.SH else np.zeros(P, np.int16)))
        # dloc: [SD chunks, P] -> [P, SD] bf16
        dloc_pc.append(np.ascontiguousarray(
            np.concatenate(d_parts, axis=0).T).astype(BF16))

    # region-reordered bf16 feature tables (256B rows)
    xpad = np.zeros((cfg.n_pad, ROW), dtype=BF16)
    xpad[: cfg.n_nodes, : cfg.in_dim] = np.asarray(x, np.float32)
    x3 = xpad.reshape(N_CORES, cfg.span, ROW)
    tab_a = np.ascontiguousarray(x3[:, : cfg.hs].reshape(cfg.reg, ROW))
    tab_b = np.ascontiguousarray(x3[:, cfg.hs:].reshape(cfg.reg, ROW))

    # per-core transposed x slice for the self (lin_r) term
    xt_pc = [
        np.ascontiguousarray(xpad[c * cfg.span:(c + 1) * cfg.span,
                                  : cfg.in_dim].T)
        for c in range(N_CORES)
    ]

    sgn = lambda w: np.sign(np.asarray(w, dtype=np.float32))
    w1lt = np.concatenate([sgn(w1_l).T, np.asarray(b1, np.float32)[None, :]],
                          0).astype(BF16)
    w1rt = np.ascontiguousarray(sgn(w1_r).T).astype(BF16)
    w2lt = np.ascontiguousarray(sgn(w2_l).T).astype(BF16)
    w2rt = np.ascontiguousarray(sgn(w2_r).T).astype(BF16)
    ib2 = np.concatenate(
        [np.eye(cfg.out_dim, dtype=np.float32),
         np.asarray(b2, np.float32)[None, :]], 0).astype(BF16)

    in_maps = []
    for c in range(N_CORES):
        in_maps.append({
            "taba": tab_a, "tabb": tab_b,
            "xt": xt_pc[c],
            "idxlo": idxlo_pc[c], "idxhi": idxhi_pc[c],
            "dloc": dloc_pc[c], "rdeg": rdeg_pc[c],
            "w1lt": w1lt, "w1rt": w1rt, "w2lt": w2lt, "w2rt": w2rt, "ib2": ib2,
        })
    return in_maps, sched


def build_program(cfg, sched, enable_asserts=False):
    tpc = cfg.tiles_per_core
    NBUF = 14                                     # rotating gather-call buffers
    NB = 3                                        # small persistent buffer depth
    SL, SH, SD = sched.SL, sched.SH, sched.SD
    KMAX = sched.KMAX

    dt = mybir.dt
    f32, bf, i16 = dt.float32, dt.bfloat16, dt.int16
    IN, HID, OUT = cfg.in_dim, cfg.hid, cfg.out_dim

    nc = bacc.Bacc("TRN2", target_bir_lowering=False, debug=False,
                   enable_asserts=enable_asserts, num_devices=N_CORES,
                   num_swdge_queues=NQ)

    taba = nc.dram_tensor("taba", [cfg.reg, ROW], bf, kind="ExternalInput")
    tabb = nc.dram_tensor("tabb", [cfg.reg, ROW], bf, kind="ExternalInput")
    xt = nc.dram_tensor("xt", [IN, cfg.span], bf, kind="ExternalInput")
    idxlo = nc.dram_tensor("idxlo", [P, SL * 8], i16, kind="ExternalInput")
    idxhi = nc.dram_tensor("idxhi", [P, max(SH, 1) * 8], i16,
                           kind="ExternalInput")
    dloc = nc.dram_tensor("dloc", [P, SD], bf, kind="ExternalInput")
    rdeg = nc.dram_tensor("rdeg", [cfg.in_dim, cfg.span], bf,
                          kind="ExternalInput")
    w1lt = nc.dram_tensor("w1lt", [IN + 1, HID], bf, kind="ExternalInput")
    w1rt = nc.dram_tensor("w1rt", [IN, HID], bf, kind="ExternalInput")
    w2lt = nc.dram_tensor("w2lt", [HID, OUT], bf, kind="ExternalInput")
    w2rt = nc.dram_tensor("w2rt", [HID, OUT], bf, kind="ExternalInput")
    ib2 = nc.dram_tensor("ib2", [OUT + 1, OUT], bf, kind="ExternalInput")
    outd = nc.dram_tensor("out", [cfg.span, OUT], f32, kind="ExternalOutput")

    AF = mybir.ActivationFunctionType
    OP = mybir.AluOpType

    with tile.TileContext(nc) as tc:
        with tc.tile_pool(name="res", bufs=1) as res, \
             tc.tile_pool(name="msgp", bufs=1) as msgp, \
             tc.tile_pool(name="ohp", bufs=3) as ohp, \
             tc.tile_pool(name="xtp", bufs=3) as xtp, \
             tc.tile_pool(name="scp", bufs=3) as scp, \
             tc.tile_pool(name="ps_agg", bufs=2, space="PSUM") as ps_agg, \
             tc.tile_pool(name="ps_o", bufs=2, space="PSUM") as ps_o, \
             tc.tile_pool(name="ps_y", bufs=2, space="PSUM") as ps_y, \
             tc.tile_pool(name="dramp", bufs=1, space="DRAM") as dramp:

            # ---------------- resident data ----------------
            idxlo_sb = res.tile([P, SL * 8], i16, name="idxlo_sb")
            nc.sync.dma_start(idxlo_sb[:], idxlo[:])
            idxhi_sb = res.tile([P, max(SH, 1) * 8], i16, name="idxhi_sb")
            nc.sync.dma_start(idxhi_sb[:], idxhi[:])
            dloc_sb = res.tile([P, SD], bf, name="dloc_sb")
            nc.sync.dma_start(dloc_sb[:], dloc[:])
            rdeg_sb = res.tile([cfg.in_dim, cfg.span], bf, name="rdeg_sb")
            nc.sync.dma_start(rdeg_sb[:], rdeg[:])
            w1lt_sb = res.tile([IN + 1, HID], bf, name="w1lt_sb")
            nc.sync.dma_start(w1lt_sb[:], w1lt[:])
            w1rt_sb = res.tile([IN, HID], bf, name="w1rt_sb")
            nc.sync.dma_start(w1rt_sb[:], w1rt[:])
            w2lt_sb = res.tile([HID, OUT], bf, name="w2lt_sb")
            nc.sync.dma_start(w2lt_sb[:], w2lt[:])
            w2rt_sb = res.tile([HID, OUT], bf, name="w2rt_sb")
            nc.sync.dma_start(w2rt_sb[:], w2rt[:])
            ib2_sb = res.tile([OUT + 1, OUT], bf, name="ib2_sb")
            nc.sync.dma_start(ib2_sb[:], ib2[:])

            iota_i = res.tile([P, P], mybir.dt.int32, name="iota_i")
            nc.gpsimd.iota(iota_i[:], pattern=[[1, P]], base=0,
                           channel_multiplier=0)
            iota_bf = res.tile([P, P], bf, name="iota_bf")
            nc.vector.tensor_copy(iota_bf[:], iota_i[:])

            ht_tiles = [res.tile([HID, P], bf, name=f"ht{t}")
                        for t in range(tpc)]

            # persistent gather-call buffers (m_lo/m_hi shared by layer 1 and
            # phase 2b; m_lo2 is separate so phase 2a can overlap layer 1)
            m_lo = [msgp.tile([P, GC, ROW], bf, name=f"mlo{i}")
                    for i in range(NBUF)]
            m_hi = [msgp.tile([P, GC, ROW], bf, name=f"mhi{i}")
                    for i in range(NBUF)]
            m_lo2 = [msgp.tile([P, GC, ROW], bf, name=f"mlo2_{i}")
                     for i in range(NBUF)]
            # persistent agg tiles (aggs1 carries the all-ones bias row)
            aggs1 = [msgp.tile([IN + 1, P], bf, name=f"aggs1_{i}")
                     for i in range(NB)]
            aggs2 = [msgp.tile([OUT, P], bf, name=f"aggs2_{i}")
                     for i in range(NB)]
            y2sbs = [msgp.tile([P, ROW], bf, name=f"y2sb{i}")
                     for i in range(NB)]
            for i in range(NB):
                nc.vector.memset(aggs1[i][IN:IN + 1, :], 1.0)
                nc.vector.memset(y2sbs[i][:, OUT:ROW], 0.0)
            # per-tile lo-class partial means for layer 2 (ones row for bias)
            pl_tiles = [msgp.tile([OUT + 1, P], bf, name=f"pl{t}")
                        for t in range(tpc)]
            for t in range(tpc):
                nc.vector.memset(pl_tiles[t][0:OUT, :], 0.0)
                nc.vector.memset(pl_tiles[t][OUT:OUT + 1, :], 1.0)

            y2in = dramp.tile([cfg.span, ROW], bf, name="y2in")
            y2fa = dramp.tile([cfg.reg, ROW], bf, name="y2fa",
                              addr_space="Shared")
            y2fb = dramp.tile([cfg.reg, ROW], bf, name="y2fb",
                              addr_space="Shared")

            qctr = [0]                            # SWDGE queue round-robin

            def make_stream(bufs, idx_sb, tab, total):
                state = [0]

                def ensure(upto_call):
                    while state[0] <= upto_call:
                        c = state[0]
                        ncall = min(GC, total - c * GC)
                        num = ncall * P
                        dest = bufs[c % NBUF]
                        nc.gpsimd.dma_gather(
                            out_ap=dest[:, 0:ncall, :],
                            in_ap=tab,
                            idxs_ap=idx_sb[:, c * (GC * 8):
                                           c * (GC * 8) + num // 16],
                            num_idxs=num,
                            num_idxs_reg=num,
                            elem_size=ROW,
                            queue_num=qctr[0] % NQ,
                        )
                        qctr[0] += 1
                        state[0] += 1
                return ensure

            def build_oh(oh, col0, n):
                """Batched one-hots for n chunk columns starting at col0:
                oh[:, j, :] = (iota == dloc_j)."""
                iota_b = iota_bf[:].unsqueeze(1).broadcast_to([P, n, P])
                dloc_b = dloc_sb[:, col0:col0 + n].unsqueeze(2) \
                    .broadcast_to([P, n, P])
                nc.vector.tensor_tensor(oh[:, 0:n, :], iota_b, dloc_b,
                                        OP.is_equal)

            def mean_scale(out_ap, agg, F_agg, t):
                """out = agg * (1/deg): one DVE mult against the resident
                pre-broadcast rdeg slab."""
                nc.vector.tensor_tensor(
                    out_ap, agg[:], rdeg_sb[0:F_agg, t * P:(t + 1) * P],
                    OP.mult)

            off_lo, off_hi = sched.off_lo, sched.off_hi
            eff_l, eff_h = sched.eff_kl, sched.eff_kh
            off_d = sched.off_d

            def call_hi(arr_off, arr_eff, t):
                return (int(arr_off[t]) + int(arr_eff[t]) - 1) // GC

            # ---------------- layer 1 (+ y2 projection) ----------------
            ens_lo1 = make_stream(m_lo, idxlo_sb, taba[:], SL)
            ens_hi1 = make_stream(m_hi, idxhi_sb, tabb[:], SH)
            for t in range(tpc):
                tp = min(t + 1, tpc - 1)           # prefetch one tile ahead
                if SL:
                    ens_lo1(call_hi(off_lo, eff_l, tp))
                if SH:
                    ens_hi1(call_hi(off_hi, eff_h, tp))
                nl, nh = int(eff_l[t]), int(eff_h[t])
                oh = ohp.tile([P, KMAX, P], bf, tag="oh")
                build_oh(oh, int(off_d[t]), nl + nh)
                agg = ps_agg.tile([IN, P], f32, tag="agg")
                chunks = [(m_lo, int(off_lo[t]) + k) for k in range(nl)]
                chunks += [(m_hi, int(off_hi[t]) + k) for k in range(nh)]
                for j, (bufs, cpos) in enumerate(chunks):
                    mb = bufs[(cpos // GC) % NBUF]
                    nc.tensor.matmul(
                        out=agg[:], lhsT=mb[:, cpos % GC, 0:IN],
                        rhs=oh[:, j, :], start=(j == 0),
                        stop=(j == len(chunks) - 1))
                ab = aggs1[t % NB]
                mean_scale(ab[0:IN, :], agg, IN, t)
                # tail: h = relu(w1l @ [agg;1] + w1r @ x_self); y2 = h @ w2l
                xt_t = xtp.tile([IN, P], bf, tag="xt")
                nc.sync.dma_start(xt_t[:], xt[:, t * P:(t + 1) * P])
                hps = ps_o.tile([HID, P], f32, tag="hps")
                nc.tensor.matmul(out=hps[:], lhsT=w1lt_sb[:], rhs=ab[:],
                                 start=True, stop=False)
                nc.tensor.matmul(out=hps[:], lhsT=w1rt_sb[:], rhs=xt_t[:],
                                 start=False, stop=True)
                nc.scalar.activation(out=ht_tiles[t][:], in_=hps[:],
                                     func=AF.Relu)
                y2ps = ps_y.tile([P, OUT], f32, tag="y2ps")
                nc.tensor.matmul(out=y2ps[:], lhsT=ht_tiles[t][:],
                                 rhs=w2lt_sb[:], start=True, stop=True)
                ysb = y2sbs[t % NB]
                nc.scalar.activation(out=ysb[:, 0:OUT], in_=y2ps[:],
                                     func=AF.Copy)
                nc.sync.dma_start(y2in[t * P:(t + 1) * P, :], ysb[:])

            # region-A AllGather after the layer-1 stream; region B fires
            # mid-phase-2a so its latency overlaps 2a work
            nc.gpsimd.collective_compute(
                "AllGather", OP.bypass,
                replica_groups=[list(range(N_CORES))],
                ins=[y2in[0:cfg.hs, :].opt()], outs=[y2fa.opt()],
            )

            # ------------- layer 2a: lo-class partial means --------------
            # only needs region A; AG#2 (region B) is emitted mid-phase so
            # its Pool block overlaps 2a compute
            ens_lo2 = make_stream(m_lo2, idxlo_sb, y2fa[:], SL)
            for t in range(tpc):
                tp = min(t + 1, tpc - 1)
                if SL:
                    ens_lo2(call_hi(off_lo, eff_l, tp))
                if t == tpc // 2:
                    nc.gpsimd.collective_compute(
                        "AllGather", OP.bypass,
                        replica_groups=[list(range(N_CORES))],
                        ins=[y2in[cfg.hs:cfg.span, :].opt()],
                        outs=[y2fb.opt()],
                    )
                nl = int(eff_l[t])
                if nl == 0:
                    continue
                oh = ohp.tile([P, KMAX, P], bf, tag="oh")
                build_oh(oh, int(off_d[t]), nl)
                agg = ps_agg.tile([OUT, P], f32, tag="agg")
                for k in range(nl):
                    cpos = int(off_lo[t]) + k
                    mb = m_lo2[(cpos // GC) % NBUF]
                    nc.tensor.matmul(
                        out=agg[:], lhsT=mb[:, cpos % GC, 0:OUT],
                        rhs=oh[:, k, :], start=(k == 0), stop=(k == nl - 1))
                mean_scale(pl_tiles[t][0:OUT, :], agg, OUT, t)

            # ------------- layer 2b: hi-class + combine + output ---------
            ens_hi2 = make_stream(m_hi, idxhi_sb, y2fb[:], SH)
            for t in range(tpc):
                tp = min(t + 1, tpc - 1)
                if SH:
                    ens_hi2(call_hi(off_hi, eff_h, tp))
                nh = int(eff_h[t])
                ops_ = ps_o.tile([P, OUT], f32, tag="hps")
                nc.tensor.matmul(out=ops_[:], lhsT=ht_tiles[t][:],
                                 rhs=w2rt_sb[:], start=True, stop=False)
                if nh:
                    oh = ohp.tile([P, KMAX, P], bf, tag="oh")
                    build_oh(oh, int(off_d[t]) + int(eff_l[t]), nh)
                    agg = ps_agg.tile([OUT, P], f32, tag="agg")
                    for k in range(nh):
                        cpos = int(off_hi[t]) + k
                        mb = m_hi[(cpos // GC) % NBUF]
                        nc.tensor.matmul(
                            out=agg[:], lhsT=mb[:, cpos % GC, 0:OUT],
                            rhs=oh[:, k, :], start=(k == 0),
                            stop=(k == nh - 1))
                    ab = aggs2[t % NB]
                    mean_scale(ab[:], agg, OUT, t)
                    nc.tensor.matmul(out=ops_[:], lhsT=ab[:],
                                     rhs=ib2_sb[0:OUT, :],
                                     start=False, stop=False)
                nc.tensor.matmul(out=ops_[:], lhsT=pl_tiles[t][:],
                                 rhs=ib2_sb[:], start=False, stop=True)
                osb = scp.tile([P, OUT], f32, tag="osb")
                nc.scalar.activation(out=osb[:], in_=ops_[:], func=AF.Copy)
                nc.sync.dma_start(outd[t * P:(t + 1) * P, :], osb[:])

    nc.compile()
    return nc


def run(inputs, cfg, trace=False):
    in_maps, sched = preprocess(cfg=cfg, **inputs)
    nc = build_program(cfg, sched)
    res = bass_utils.run_bass_kernel_spmd(
        nc, in_maps, list(range(N_CORES)), trace=trace)
    outs = [res.results[c]["out"] for c in range(N_CORES)]
    full = np.concatenate(outs, axis=0)[: cfg.n_nodes]
    return np.ascontiguousarray(full.astype(np.float32)), res


def kernel(**inputs):
    out, _ = run(inputs, FULL_CFG, trace=False)
    return out
